# revision 31
# baseline (speedup 1.0000x reference)
"""Trainium2 Bass kernel for nn_DecentLayer (gnn_message_passing).

The reference gathers 16 of 24 input channels via static position matching,
then runs a 3x3 same-padded conv: [B=16, 16, 256, 256] x [32, 16, 3, 3]
-> [B, 32, 256, 256].

Strategy (v2):
  * Data-parallel over batch: 8 cores x 2 images ("phases").
  * Host pre-assembles the SBUF-ready input: per image, 8 horizontal strips
    of 32 output rows in a zero-padded row-major layout (258 cols x 34 rows
    incl. halos), pre-cast to bf16. Each strip occupies 16 partitions for
    the unshifted copy plus 16 partitions holding the same rows shifted by
    one column ("copy1"). Partition p = 32*gg + 16*cp + ch; strip slot
    = 2*gg + sg with sg indexed along the free dim. One contiguous DMA per
    phase -- full-rate descriptors, no on-chip padding or casts.
  * Conv = shifted matmuls accumulating in PSUM. K=128 block-diagonal
    stationary batches 4 strips x (16 ch x 2 copies); M=128 = 4 strips x 32
    filters. The shifted copy turns two horizontal taps into ONE matmul:
    per output row, 3 pair-matmuls (dw=0,1) + 3 single-matmuls (dw=2)
    instead of 9. All tap shifts are SBUF address offsets.
  * PSUM: one output row (N=256) per bank, 8 banks rotating; DVE evacuates
    to an output stage; SWDGE DMA (all 16 engines) stores to HBM.
"""

import numpy as np
import ml_dtypes

import concourse.bass as bass
import concourse.bacc as bacc
import concourse.mybir as mybir
import concourse.tile as tile
from concourse.bass_utils import run_bass_kernel_spmd

# Problem constants (hardcoded per the harness contract).
N_CORES = 8
B = 16
IMGS_PER_CORE = B // N_CORES  # 2
CIN = 16      # conv input channels after gather
COUT = 32     # filters
H = W = 256
SLOTS = 8     # strips per image
HS = H // SLOTS   # 32 output rows per strip
ROWS = HS + 2     # strip rows incl. halo
WP = W + 2        # padded row width
SSTRIDE = ROWS * WP  # 8772 elems per strip per partition
HALF = 8          # output rows per store chunk
N_TAPMM = 6       # matmuls per output row: 3 pairs + 3 singles

MODE = "bf16"  # "bf16" or "f32r" (fp32 storage streamed as float32r)


def _common_pairs(ms_in, ns_in, ms_x, ns_x):
    ms_in = np.asarray(ms_in)
    ns_in = np.asarray(ns_in)
    ms_x = np.asarray(ms_x)
    ns_x = np.asarray(ns_x)
    f_ids, x_ids = [], []
    for i_in in range(ms_in.shape[0]):
        hits = np.nonzero((ms_x == ms_in[i_in]) & (ns_x == ns_in[i_in]))[0]
        for i_x in hits:
            f_ids.append(i_in)
            x_ids.append(int(i_x))
    return np.asarray(f_ids), np.asarray(x_ids)


def build_program(n_img=IMGS_PER_CORE, mode=MODE):
    """Build the per-core Bass program. Returns compiled Bacc."""
    f32 = mybir.dt.float32
    if mode == "f32r":
        sb_dt, dram_dt = mybir.dt.float32r, f32  # DMA cast rounds to f32r
    else:
        sb_dt, dram_dt = mybir.dt.bfloat16, mybir.dt.bfloat16

    nc = bacc.Bacc("TRN2", target_bir_lowering=False, debug=False)
    x_in = nc.dram_tensor("x", [n_img, 2, 128, SSTRIDE], dram_dt,
                          kind="ExternalInput")
    w_in = nc.dram_tensor("w", [128, N_TAPMM, 128], dram_dt,
                          kind="ExternalInput")
    y_out = nc.dram_tensor("y", [n_img, COUT, H, W], f32, kind="ExternalOutput")

    # h_abs = 64*gg + 32*sg + 16*m + r   (strip slot = 2*gg + sg)
    y_r = y_out[:].rearrange(
        "b co (gg sg m r) w -> b sg m gg co r w", gg=4, sg=2, m=HS // HALF, r=HALF
    )

    with tile.TileContext(nc) as tc:
        with (
            tc.tile_pool(name="persist", bufs=1) as persist,
            tc.tile_pool(name="op", bufs=5) as op,
            tc.tile_pool(name="ps", bufs=8, space="PSUM") as psp,
        ):
            wt = persist.tile([128, N_TAPMM, 128], sb_dt, name="wt")
            nc.gpsimd.dma_start(out=wt[:], in_=w_in[:])

            # Per-(phase, sg) input, split into two overlapping row-range
            # tiles: A = strip rows [0, 18) serves row-pairs h <= 14,
            # B = rows [16, 34) serves h >= 16 (each pair h reads rows
            # h..h+3). Finer dependencies let the first matmuls start after
            # ~half the input lands; the 2-row overlap re-reads ~6% of HBM.
            A_ROWS, B_START = 18, 16
            xbufs = {}
            for p in range(n_img):
                for sg in range(2):
                    xa = persist.tile([128, A_ROWS * WP], sb_dt,
                                      name=f"xa{p}{sg}")
                    xz = persist.tile([128, (ROWS - B_START) * WP], sb_dt,
                                      name=f"xz{p}{sg}")
                    xbufs[p, sg] = (xa, xz)
                    nc.gpsimd.dma_start(
                        out=xa[:], in_=x_in[p, sg][:, : A_ROWS * WP]
                    )
                    nc.gpsimd.dma_start(
                        out=xz[:], in_=x_in[p, sg][:, B_START * WP :]
                    )

            # Small frequent store chunks keep several DMAs outstanding on
            # the SWDGE ring (all 16 engines), which raises sustained store
            # bandwidth and shrinks the end-of-kernel drain backlog.
            n_dma = 0
            n_units = 2 * n_img
            for p in range(n_img):
                for sg in range(2):
                    xa, xz = xbufs[p, sg]
                    xva = xa[:].rearrange("q (r c) -> q r c", c=WP)
                    xvz = xz[:].rearrange("q (r c) -> q r c", c=WP)
                    last_unit = p == n_img - 1 and sg == 1
                    outt = None
                    for h in range(0, HS, 2):  # two output rows per matmul
                        m, r = divmod(h, HALF)
                        if h + 3 < A_ROWS:
                            xv, hl = xva, h
                        else:
                            xv, hl = xvz, h - B_START
                        ps = psp.tile([128, 2 * W], f32, name="acc")
                        for t in range(N_TAPMM):
                            dh, dw0 = t % 3, (0 if t < 3 else 2)
                            nc.tensor.matmul(
                                ps[:],
                                wt[:, t, :],
                                xv[:, hl + dh : hl + dh + 2, dw0 : dw0 + W],
                                start=(t == 0),
                                stop=(t == N_TAPMM - 1),
                            )
                        if r == 0:
                            outt = op.tile([128, HALF * W], f32, name="ot")
                        nc.vector.tensor_copy(outt[:, r * W : (r + 2) * W], ps[:])
                        if r == HALF - 2:
                            # route the final chunks to the idle HWDGE rings
                            # so the drain after the last matmul parallelizes
                            if last_unit and m == HS // HALF - 1:
                                eng = nc.sync
                            elif last_unit and m == HS // HALF - 2:
                                eng = nc.scalar
                            else:
                                eng = nc.gpsimd
                            eng.dma_start(out=y_r[p, sg, m], in_=outt[:])
                            n_dma += 1

    nc.compile()
    return nc


_NC_CACHE = {}


def _get_program(mode=MODE):
    if mode not in _NC_CACHE:
        _NC_CACHE[mode] = build_program(mode=mode)
    return _NC_CACHE[mode]


def _host_prep(inputs):
    x = np.asarray(inputs["x_data"], dtype=np.float32)
    w = np.asarray(inputs["weights"], dtype=np.float32)
    f_ids, x_ids = _common_pairs(
        inputs["ms_in"], inputs["ns_in"], inputs["ms_x"], inputs["ns_x"]
    )
    assert len(f_ids) == CIN, f"expected {CIN} matched pairs, got {len(f_ids)}"
    xg = x[:, x_ids]                                 # [B, 16, H, W]
    wg = w[:, f_ids]                                 # [COUT, 16, 3, 3]

    np_dt = ml_dtypes.bfloat16 if MODE == "bf16" else np.float32
    xc = xg.astype(np_dt)

    # SBUF-ready layout: [B, 128, 2, ROWS, WP]; partition = 32*gg + 16*cp + ch,
    # strip slot = 2*gg + sg; copy cp=1 holds the same rows shifted one column
    # left (value at col c = padded col c+1) so one matmul covers taps
    # (dh, dw) and (dh, dw+1).
    host = np.zeros((B, 128, 2, ROWS, WP), dtype=np_dt)
    for slot in range(SLOTS):
        gg, sg = divmod(slot, 2)
        r_lo = max(0, HS * slot - 1)
        r_hi = min(H, HS * slot + HS + 1)
        dst_r0 = r_lo - (HS * slot - 1)
        n = r_hi - r_lo
        rows = xc[:, :, r_lo:r_hi, :]
        p0 = 32 * gg
        host[:, p0 : p0 + 16, sg, dst_r0 : dst_r0 + n, 1 : W + 1] = rows
        host[:, p0 + 16 : p0 + 32, sg, dst_r0 : dst_r0 + n, 0:W] = rows
    # -> [B, sg, 128, SSTRIDE] so each (phase, sg) load is one contiguous DMA
    host = np.ascontiguousarray(
        host.reshape(B, 128, 2, SSTRIDE).transpose(0, 2, 1, 3)
    )

    # Stationaries [128, 6, 128]: t in 0..2 = pair (W[dh,0] | W[dh,1]),
    # t in 3..5 = single (W[dh,2] | 0). Block-diagonal over 4 strips.
    w_host = np.zeros((128, N_TAPMM, 128), dtype=np.float32)
    for dh in range(3):
        for gg in range(4):
            q = 32 * gg
            w_host[q : q + 16, dh, q : q + 32] = wg[:, :, dh, 0].T
            w_host[q + 16 : q + 32, dh, q : q + 32] = wg[:, :, dh, 1].T
            w_host[q : q + 16, 3 + dh, q : q + 32] = wg[:, :, dh, 2].T
    w_host = w_host.astype(np_dt)
    return host, w_host


def _run(inputs, trace=False):
    xh, w_host = _host_prep(inputs)
    nc = _get_program()
    in_maps = [
        {"x": xh[IMGS_PER_CORE * k : IMGS_PER_CORE * (k + 1)], "w": w_host}
        for k in range(N_CORES)
    ]
    res = run_bass_kernel_spmd(nc, in_maps, list(range(N_CORES)), trace=trace)
    out = np.concatenate([r["y"] for r in res.results], axis=0)
    return out, res


def kernel(**inputs):
    out, _ = _run(inputs, trace=False)
    return out


# revision 33
# speedup vs baseline: 1.0370x; 1.0370x over previous
"""Trainium2 Bass kernel for nn_DecentLayer (gnn_message_passing).

The reference gathers 16 of 24 input channels via static position matching,
then runs a 3x3 same-padded conv: [B=16, 16, 256, 256] x [32, 16, 3, 3]
-> [B, 32, 256, 256].

Strategy (v2):
  * Data-parallel over batch: 8 cores x 2 images ("phases").
  * Host pre-assembles the SBUF-ready input: per image, 8 horizontal strips
    of 32 output rows in a zero-padded row-major layout (258 cols x 34 rows
    incl. halos), pre-cast to bf16. Each strip occupies 16 partitions for
    the unshifted copy plus 16 partitions holding the same rows shifted by
    one column ("copy1"). Partition p = 32*gg + 16*cp + ch; strip slot
    = 2*gg + sg with sg indexed along the free dim. One contiguous DMA per
    phase -- full-rate descriptors, no on-chip padding or casts.
  * Conv = shifted matmuls accumulating in PSUM. K=128 block-diagonal
    stationary batches 4 strips x (16 ch x 2 copies); M=128 = 4 strips x 32
    filters. The shifted copy turns two horizontal taps into ONE matmul:
    per output row, 3 pair-matmuls (dw=0,1) + 3 single-matmuls (dw=2)
    instead of 9. All tap shifts are SBUF address offsets.
  * PSUM: one output row (N=256) per bank, 8 banks rotating; DVE evacuates
    to an output stage; SWDGE DMA (all 16 engines) stores to HBM.
"""

import numpy as np
import ml_dtypes

import concourse.bass as bass
import concourse.bacc as bacc
import concourse.mybir as mybir
import concourse.tile as tile
from concourse.bass_utils import run_bass_kernel_spmd

# Problem constants (hardcoded per the harness contract).
N_CORES = 8
B = 16
IMGS_PER_CORE = B // N_CORES  # 2
CIN = 16      # conv input channels after gather
COUT = 32     # filters
H = W = 256
SLOTS = 8     # strips per image
HS = H // SLOTS   # 32 output rows per strip
ROWS = HS + 2     # strip rows incl. halo
WP = W + 2        # padded row width
SSTRIDE = ROWS * WP  # 8772 elems per strip per partition
HALF = 8          # output rows per store chunk
N_TAPMM = 6       # matmuls per output row: 3 pairs + 3 singles

MODE = "bf16"  # "bf16" or "f32r" (fp32 storage streamed as float32r)


def _common_pairs(ms_in, ns_in, ms_x, ns_x):
    ms_in = np.asarray(ms_in)
    ns_in = np.asarray(ns_in)
    ms_x = np.asarray(ms_x)
    ns_x = np.asarray(ns_x)
    f_ids, x_ids = [], []
    for i_in in range(ms_in.shape[0]):
        hits = np.nonzero((ms_x == ms_in[i_in]) & (ns_x == ns_in[i_in]))[0]
        for i_x in hits:
            f_ids.append(i_in)
            x_ids.append(int(i_x))
    return np.asarray(f_ids), np.asarray(x_ids)


def build_program(n_img=IMGS_PER_CORE, mode=MODE):
    """Build the per-core Bass program. Returns compiled Bacc."""
    f32 = mybir.dt.float32
    if mode == "f32r":
        sb_dt, dram_dt = mybir.dt.float32r, f32  # DMA cast rounds to f32r
    else:
        sb_dt, dram_dt = mybir.dt.bfloat16, mybir.dt.bfloat16

    nc = bacc.Bacc("TRN2", target_bir_lowering=False, debug=False)
    x_in = nc.dram_tensor("x", [n_img, 2, 128, SSTRIDE], dram_dt,
                          kind="ExternalInput")
    w_in = nc.dram_tensor("w", [128, N_TAPMM, 128], dram_dt,
                          kind="ExternalInput")
    y_out = nc.dram_tensor("y", [n_img, COUT, H, W], f32, kind="ExternalOutput")

    # h_abs = 64*gg + 32*sg + 16*m + r   (strip slot = 2*gg + sg)
    y_r = y_out[:].rearrange(
        "b co (gg sg m r) w -> b sg m gg co r w", gg=4, sg=2, m=HS // HALF, r=HALF
    )

    with tile.TileContext(nc) as tc:
        with (
            tc.tile_pool(name="persist", bufs=1) as persist,
            tc.tile_pool(name="op", bufs=5) as op,
            tc.tile_pool(name="ps", bufs=8, space="PSUM") as psp,
        ):
            wt = persist.tile([128, N_TAPMM, 128], sb_dt, name="wt")
            nc.gpsimd.dma_start(out=wt[:], in_=w_in[:])

            # Per-(phase, sg) input, split into two overlapping row-range
            # tiles: A = strip rows [0, 18) serves row-pairs h <= 14,
            # B = rows [16, 34) serves h >= 16 (each pair h reads rows
            # h..h+3). Finer dependencies let the first matmuls start after
            # ~half the input lands; the 2-row overlap re-reads ~6% of HBM.
            A_ROWS, B_START = 18, 16
            xbufs = {}
            for p in range(n_img):
                for sg in range(2):
                    xa = persist.tile([128, A_ROWS * WP], sb_dt,
                                      name=f"xa{p}{sg}")
                    xz = persist.tile([128, (ROWS - B_START) * WP], sb_dt,
                                      name=f"xz{p}{sg}")
                    xbufs[p, sg] = (xa, xz)
                    nc.gpsimd.dma_start(
                        out=xa[:], in_=x_in[p, sg][:, : A_ROWS * WP]
                    )
                    nc.gpsimd.dma_start(
                        out=xz[:], in_=x_in[p, sg][:, B_START * WP :]
                    )

            # Small frequent store chunks keep several DMAs outstanding on
            # the SWDGE ring (all 16 engines), which raises sustained store
            # bandwidth and shrinks the end-of-kernel drain backlog.
            n_dma = 0
            n_units = 2 * n_img
            for p in range(n_img):
                for sg in range(2):
                    xa, xz = xbufs[p, sg]
                    xva = xa[:].rearrange("q (r c) -> q r c", c=WP)
                    xvz = xz[:].rearrange("q (r c) -> q r c", c=WP)
                    last_unit = p == n_img - 1 and sg == 1
                    outt = None
                    for h in range(0, HS, 2):  # two output rows per matmul
                        m, r = divmod(h, HALF)
                        if h + 3 < A_ROWS:
                            xv, hl = xva, h
                        else:
                            xv, hl = xvz, h - B_START
                        ps = psp.tile([128, 2 * W], f32, name="acc")
                        for t in range(N_TAPMM):
                            dh, dw0 = t % 3, (0 if t < 3 else 2)
                            nc.tensor.matmul(
                                ps[:],
                                wt[:, t, :],
                                xv[:, hl + dh : hl + dh + 2, dw0 : dw0 + W],
                                start=(t == 0),
                                stop=(t == N_TAPMM - 1),
                            )
                        if r == 0:
                            outt = op.tile([128, HALF * W], f32, name="ot")
                        nc.vector.tensor_copy(outt[:, r * W : (r + 2) * W], ps[:])
                        if r == HALF - 2:
                            # All stores on the SWDGE ring (16 engines).
                            # Routing any store through the HWDGE rings
                            # (4 engines, ~100 GB/s) measured strictly worse.
                            nc.gpsimd.dma_start(out=y_r[p, sg, m], in_=outt[:])
                            n_dma += 1

    nc.compile()
    return nc


_NC_CACHE = {}


def _get_program(mode=MODE):
    if mode not in _NC_CACHE:
        _NC_CACHE[mode] = build_program(mode=mode)
    return _NC_CACHE[mode]


def _host_prep(inputs):
    x = np.asarray(inputs["x_data"], dtype=np.float32)
    w = np.asarray(inputs["weights"], dtype=np.float32)
    f_ids, x_ids = _common_pairs(
        inputs["ms_in"], inputs["ns_in"], inputs["ms_x"], inputs["ns_x"]
    )
    assert len(f_ids) == CIN, f"expected {CIN} matched pairs, got {len(f_ids)}"
    xg = x[:, x_ids]                                 # [B, 16, H, W]
    wg = w[:, f_ids]                                 # [COUT, 16, 3, 3]

    np_dt = ml_dtypes.bfloat16 if MODE == "bf16" else np.float32
    xc = xg.astype(np_dt)

    # SBUF-ready layout: [B, 128, 2, ROWS, WP]; partition = 32*gg + 16*cp + ch,
    # strip slot = 2*gg + sg; copy cp=1 holds the same rows shifted one column
    # left (value at col c = padded col c+1) so one matmul covers taps
    # (dh, dw) and (dh, dw+1).
    host = np.zeros((B, 128, 2, ROWS, WP), dtype=np_dt)
    for slot in range(SLOTS):
        gg, sg = divmod(slot, 2)
        r_lo = max(0, HS * slot - 1)
        r_hi = min(H, HS * slot + HS + 1)
        dst_r0 = r_lo - (HS * slot - 1)
        n = r_hi - r_lo
        rows = xc[:, :, r_lo:r_hi, :]
        p0 = 32 * gg
        host[:, p0 : p0 + 16, sg, dst_r0 : dst_r0 + n, 1 : W + 1] = rows
        host[:, p0 + 16 : p0 + 32, sg, dst_r0 : dst_r0 + n, 0:W] = rows
    # -> [B, sg, 128, SSTRIDE] so each (phase, sg) load is one contiguous DMA
    host = np.ascontiguousarray(
        host.reshape(B, 128, 2, SSTRIDE).transpose(0, 2, 1, 3)
    )

    # Stationaries [128, 6, 128]: t in 0..2 = pair (W[dh,0] | W[dh,1]),
    # t in 3..5 = single (W[dh,2] | 0). Block-diagonal over 4 strips.
    w_host = np.zeros((128, N_TAPMM, 128), dtype=np.float32)
    for dh in range(3):
        for gg in range(4):
            q = 32 * gg
            w_host[q : q + 16, dh, q : q + 32] = wg[:, :, dh, 0].T
            w_host[q + 16 : q + 32, dh, q : q + 32] = wg[:, :, dh, 1].T
            w_host[q : q + 16, 3 + dh, q : q + 32] = wg[:, :, dh, 2].T
    w_host = w_host.astype(np_dt)
    return host, w_host


def _run(inputs, trace=False):
    xh, w_host = _host_prep(inputs)
    nc = _get_program()
    in_maps = [
        {"x": xh[IMGS_PER_CORE * k : IMGS_PER_CORE * (k + 1)], "w": w_host}
        for k in range(N_CORES)
    ]
    res = run_bass_kernel_spmd(nc, in_maps, list(range(N_CORES)), trace=trace)
    out = np.concatenate([r["y"] for r in res.results], axis=0)
    return out, res


def kernel(**inputs):
    out, _ = _run(inputs, trace=False)
    return out
